# revision 2
# baseline (speedup 1.0000x reference)
"""Trainium2 Bass kernel for nn_CapsuleLayer_46677704573208.

Math note
---------
The reference's dynamic-routing update is degenerate:
    change = sum(outputs * probs, axis=-1)   # [B,C,R,1,1]
does not depend on u (only on outputs and probs), and in iteration 1
probs is uniform, so `change` is independent of the route index r.  By
induction logits stays constant along both r and the trailing o axis for
all three iterations, hence probs[b,c] is a per-(batch, capsule) scalar
and
    outputs = squash(probs[b,c] * S[b,c,:]),   S[b,c,o] = sum_r u[b,c,r,o].
S collapses to one dense matmul:
    S = X[B, R*I] @ W2[R*I, C*O],  W2[(r,i),(c,o)] = routing_weights[c,r,i,o]
i.e. [256, 9216] @ [9216, 160].  Everything after S is tiny [256,10,16]
elementwise math.

Sharding
--------
The contraction dim K = 9216 is sharded 8 ways (1152 rows per core): each
core reads only its x-slice + W2-slice - no replication; total HBM
traffic across the fleet equals the input size.  Each core produces
partial S sums ([256,160] per PSUM group); partials are summed on the
host (the "unshard" step) and the negligible routing epilogue is applied
there.

Perf notes (measured via NTFF traces; times rel. to the profiled window)
-----------------------------------------------------------------------
* The window = [first gpsimd const-AP memset, end of last runtime-patched
  instruction].  A fixed ~6.6us runtime postamble (per-semaphore resets,
  ~51 per engine; the PE's 115ns-cadence chain is the bound) sits inside
  the window after the output-DMA drain.  Runtime-injected, kernel code
  cannot remove it; the lever is everything BEFORE the drain.
* Inputs fp16 (halves DMA bytes; PE at 1 cycle/row).  fp8 fails the 2e-2
  gate (epilogue ~doubles input-quantization error).
* x and w k-tiles packed in ONE dram tensor [128, KT, B+CO]: one DMA
  chunk carries matched k-tiles of both operands; per-partition packet =
  ksz*832B, and >=3-ktile chunks reach the full ~25GB/s-per-engine DMA
  packet rate (16 engines/queue, ~390GB/s port shared by both rings).
* Chunks split across the sync (SP) + scalar (Act) HWDGE rings; scalar's
  first byte lags ~1.8us vs sync's ~0.6us, so sync carries ~2/3 of the
  bytes and the final chunk.
* PSUM accumulation is split into group A (all but the last chunk) and
  group B (last chunk, 1 k-tile): A's cast + 80KB output DMA hide under
  the last chunk's transfer + matmuls, so only B's tiny tail (2 matmuls,
  cast, two 40KB DMAs) sits between the last input byte and the drain.
  Host sums 16 partials instead of 8.
* 7 fp32 warm-up matmuls on (uninitialized) SBUF keep the PE busy
  ~3.7us from the body start, so the HAM clock gate lifts 1.2->2.4GHz
  right as the first chunk lands.
* CAPS2_PAD fp16 pad matmuls after the real stream keep the PE array
  active until the end-of-NEFF barrier (probing whether the runtime
  reset chain's 115ns cadence is clock-gating).
"""

import contextlib
import os

import numpy as np

import concourse.bass as bass
import concourse.mybir as mybir
from concourse import bass_utils

# Problem constants (hardcoded; harness calls kernel(**inputs) standalone).
B, R, I, C, O = 256, 1152, 8, 10, 16
N_CORES = 8
K = R * I            # 9216 total contraction length, index = r*I + i
KC = K // N_CORES    # 1152 contraction rows per core
KT = KC // 128       # 9 k-tiles of 128 per core
CO = C * O           # 160 output columns (c,o)
MT = B // 128        # 2 output row tiles of 128 batch rows
F32 = mybir.dt.float32
F16 = mybir.dt.float16
BF16 = mybir.dt.bfloat16

# k-tile group boundaries for the input DMA chunks (must sum to KT).
CHUNKS = [int(c) for c in os.environ.get("CAPS2_CHUNKS", "3,3,2,1").split(",")]
assert sum(CHUNKS) == KT
CHUNK_START = [sum(CHUNKS[:i]) for i in range(len(CHUNKS))]
NCH = len(CHUNKS)
# per-chunk DMA ring assignment (S=sync, C=scalar).
_default_rings = ",".join("C" if i == 1 else "S" for i in range(NCH))
RING_MAP = os.environ.get(
    "CAPS2_RINGS", "S,C,S,S" if NCH == 4 else _default_rings).split(",")
assert len(RING_MAP) == NCH and all(r in ("S", "C") for r in RING_MAP)
# partial-S output dtype leaving the core
OUT_DT = {"bf16": BF16, "f32": F32}[os.environ.get("CAPS2_OUT_DT", "bf16")]
# fp32 warm-up matmuls (each lowers to 2 ISA matmuls, ~267ns cold): ~3.7us
# of PE activity lifts the HAM clock gate 1.2 -> 2.4GHz as data lands.
N_WARM = int(os.environ.get("CAPS2_WARM", "7"))
# split-K PSUM: group A = chunks[:SPLITK], group B = the rest.  0 disables
# (single PSUM group, output all at the end, like v1).
SPLITK = int(os.environ.get("CAPS2_SPLITK", str(NCH - 1)))
assert 0 <= SPLITK < NCH
# fp16 pad matmuls (on garbage SBUF/PSUM) after the real stream: keep the
# PE array busy until ~the end-of-NEFF barrier release.
N_PAD = int(os.environ.get("CAPS2_PAD", "0"))
# diagnostic: N dummy sem_incs on the tensor engine after the pads, to
# measure the warm EVENT_SEMAPHORE issue cadence from the trace.
N_PROBE = int(os.environ.get("CAPS2_PROBE", "0"))

_compiled = None
last_results = None  # BassKernelResults of most recent run (for test harness)


def build():
    nc = bass.Bass("TRN2", target_bir_lowering=False, debug=False,
                   num_devices=N_CORES)
    # x and w k-tiles packed side by side: [..., 0:B] is x, [..., B:B+CO] is w
    xw_d = nc.dram_tensor("xw", [128, KT, B + CO], F16, kind="ExternalInput")
    split = SPLITK > 0
    if split:
        outa_d = nc.dram_tensor("outa", [128, MT, CO], OUT_DT,
                                kind="ExternalOutput")
        outb_d = nc.dram_tensor("outb", [128, MT, CO], OUT_DT,
                                kind="ExternalOutput")
    else:
        outa_d = nc.dram_tensor("outa", [128, MT, CO], OUT_DT,
                                kind="ExternalOutput")

    with contextlib.ExitStack() as ctx:
        s_in = [ctx.enter_context(nc.semaphore(f"s_in{c}")) for c in range(NCH)]
        s_pa = ctx.enter_context(nc.semaphore("s_pa"))
        s_pb = ctx.enter_context(nc.semaphore("s_pb"))
        s_cp = ctx.enter_context(nc.semaphore("s_cp"))
        s_out = ctx.enter_context(nc.semaphore("s_out"))
        if N_PROBE:
            s_probe = ctx.enter_context(nc.semaphore("s_probe"))
        xw = ctx.enter_context(nc.sbuf_tensor("xws", [128, KT, B + CO], F16))
        acca = ctx.enter_context(nc.psum_tensor("acca", [128, MT, 512], F32))
        oba = ctx.enter_context(nc.sbuf_tensor("oba", [128, MT, CO], OUT_DT))
        if split:
            accb = ctx.enter_context(nc.psum_tensor("accb", [128, MT, 512], F32))
            obb = ctx.enter_context(nc.sbuf_tensor("obb", [128, MT, CO], OUT_DT))
        if N_WARM or N_PAD:
            # never written: warm-up/pad matmuls run on SBUF garbage and
            # their PSUM result is never read.  No memset: gpsimd stays out
            # of the body.
            zs = ctx.enter_context(nc.sbuf_tensor("zs", [128, 160], F32))
            zps = ctx.enter_context(nc.psum_tensor("zps", [128, 160], F32))

        # ---- sync + scalar: the input chunk DMAs ----
        sync = nc.sync
        scalar = nc.scalar
        for ci in range(NCH):
            k0, ksz = CHUNK_START[ci], CHUNKS[ci]
            eng = scalar if RING_MAP[ci] == "C" else sync
            eng.dma_start(
                xw[:, k0:k0 + ksz, :],
                xw_d[:, k0:k0 + ksz, :],
            ).then_inc(s_in[ci], 16)

        # ---- output DMAs ----
        # All PSUM -> SBUF casts run on the DVE (scalar-engine copies risk
        # ACT_TABLE_LOAD stalls + sequencer-runs-ahead races with dma_start;
        # semaphore-gated triggers on sync/scalar are race-free).
        if split:
            # group A (all but the last chunk): one 80KB DMA on scalar,
            # issued mid-body while the last input chunk is still landing.
            scalar.wait_ge(s_cp, 1)
            scalar.dma_start(outa_d[:, :, :], oba[:, :, :]).then_inc(s_out, 16)
            # group B (last chunk): two 40KB half-batch DMAs, one per ring.
            scalar.wait_ge(s_cp, 2)
            scalar.dma_start(outb_d[:, 1, :], obb[:, 1, :]).then_inc(s_out, 16)
            sync.wait_ge(s_cp, 3)
            sync.dma_start(outb_d[:, 0, :], obb[:, 0, :]).then_inc(s_out, 16)
        else:
            scalar.wait_ge(s_cp, 1)
            scalar.dma_start(outa_d[:, 1, :], oba[:, 1, :]).then_inc(s_out, 16)
            sync.wait_ge(s_cp, 2)
            sync.dma_start(outa_d[:, 0, :], oba[:, 0, :]).then_inc(s_out, 16)

        # ---- tensor: warm-up + the real matmul stream ----
        tensor = nc.tensor
        if N_WARM:
            for i in range(N_WARM):
                tensor.matmul(zps[:, :], zs[:, :128], zs[:, :],
                              start=(i == 0), stop=(i == N_WARM - 1))
        for ci in range(NCH):
            acc = acca if (not split or ci < SPLITK) else accb
            grp = [c for c in range(NCH)
                   if (c < SPLITK) == (ci < SPLITK)] if split else list(range(NCH))
            first_c, last_c = grp[0], grp[-1]
            tensor.wait_ge(s_in[ci], 16)
            k0, ksz = CHUNK_START[ci], CHUNKS[ci]
            for kk in range(ksz):
                k = k0 + kk
                for t in range(MT):
                    mm = tensor.matmul(
                        acc[:, t, 0:CO],
                        xw[:, k, bass.ts(t, 128)],      # lhsT: 128 batch cols
                        xw[:, k, B:B + CO],             # rhs: CO weight cols
                        start=(ci == first_c and kk == 0),
                        stop=(ci == last_c and kk == ksz - 1),
                    )
                    if ci == last_c and kk == ksz - 1 and t == MT - 1:
                        mm.then_inc(s_pa if (not split or ci < SPLITK)
                                    else s_pb, 1)
        if N_PAD:
            # fp16 garbage matmuls (~133ns warm) keep the PE array active
            # until the end-of-NEFF barrier.
            for _ in range(N_PAD):
                tensor.matmul(zps[:, :CO], xw[:, 0, 0:128], xw[:, 0, B:B + CO],
                              start=True, stop=True)
        if N_PROBE:
            for _ in range(N_PROBE):
                tensor.sem_inc(s_probe, 1)

        # ---- vector: PSUM -> SBUF casts ----
        vector = nc.vector
        if split:
            vector.wait_ge(s_pa, 1)
            vector.tensor_copy(oba[:, 0, :], acca[:, 0, 0:CO])
            vector.tensor_copy(oba[:, 1, :], acca[:, 1, 0:CO]).then_inc(s_cp, 1)
            vector.wait_ge(s_pb, 1)
            vector.tensor_copy(obb[:, 1, :], accb[:, 1, 0:CO]).then_inc(s_cp, 1)
            vector.tensor_copy(obb[:, 0, :], accb[:, 0, 0:CO]).then_inc(s_cp, 1)
        else:
            vector.wait_ge(s_pa, 1)
            vector.tensor_copy(oba[:, 1, :], acca[:, 1, 0:CO]).then_inc(s_cp, 1)
            vector.tensor_copy(oba[:, 0, :], acca[:, 0, 0:CO]).then_inc(s_cp, 1)

    return nc


def _shard_inputs(x, w):
    # K-major matrices; K index = r*I + i so per-core r-slices are
    # contiguous row blocks.  Pack x and w k-tiles into one tensor.
    xt_full = np.ascontiguousarray(x.transpose(1, 2, 0)).reshape(K, B)
    w2_full = np.ascontiguousarray(w.transpose(1, 2, 0, 3)).reshape(K, CO)
    xw_full = np.concatenate([xt_full, w2_full], axis=1).astype(np.float16)
    in_maps = []
    for j in range(N_CORES):
        sl = xw_full[j * KC:(j + 1) * KC]                     # [1152, B+CO]
        sl = sl.reshape(KT, 128, B + CO).transpose(1, 0, 2)   # [128, KT, B+CO]
        in_maps.append({"xw": np.ascontiguousarray(sl)})
    return in_maps


def _routing_epilogue(S):
    # S: [B, C, O] fp32. Collapsed 3-iteration routing (see module docstring).
    # squash(v) = (v2/(1+v2)) * v/|v| = v*|v|/(1+v2); the second form is
    # exact for v != 0 and returns 0 (the limit) instead of NaN at v == 0,
    # which bf16-rounded partial sums can actually produce.
    def squash(v):
        return v * np.abs(v) / (1.0 + v * v)

    out = squash(S * np.float32(0.1))
    logits = np.float32(0.1) * out.sum(-1)
    for _ in range(2):
        mmax = logits.max(1, keepdims=True)
        e = np.exp(logits - mmax)
        p = e / e.sum(1, keepdims=True)
        out = squash(p[:, :, None] * S)
        logits = logits + p * out.sum(-1)
    return out


def kernel(x, routing_weights):
    global _compiled, last_results
    x = np.ascontiguousarray(np.asarray(x, dtype=np.float32))
    w = np.ascontiguousarray(np.asarray(routing_weights, dtype=np.float32))
    assert x.shape == (B, R, I) and w.shape == (C, R, I, O)

    in_maps = _shard_inputs(x, w)
    if _compiled is None:
        _compiled = build()

    trace = bool(int(os.environ.get("CAPS_KERNEL_TRACE", "0")))
    res = bass_utils.run_bass_kernel_spmd(
        _compiled, in_maps, core_ids=list(range(N_CORES)), trace=trace,
    )
    last_results = res

    # sum per-core partial S tensors ([128, MT, CO] each) in fp32 on host
    S = np.zeros((128, MT, CO), dtype=np.float32)
    for core_out in res.results:
        for v in core_out.values():
            S += np.asarray(v, dtype=np.float32)
    S = np.ascontiguousarray(S.transpose(1, 0, 2)).reshape(B, C, O)
    out = _routing_epilogue(S)
    return out.reshape(B, C, 1, 1, O).astype(np.float32)
